# revision 23
# baseline (speedup 1.0000x reference)
"""B-spline evaluation kernel for Trainium2 (8 NeuronCores, data-parallel).

Math: uniform cubic B-spline, 64 basis fns, knots linspace(0,1,68).
For s = 67*x: cell = floor(s), u = s - cell,
    y = A0[cell] + A1[cell]*u + A2[cell]*u^2 + A3[cell]*u^3
with per-cell coefficients A_q derived from coefs on host.

Device algorithm (blocked PE-matmul gather):
  A_q[cell] - A_q[0] = sum_{slot s=1..66} w[s,q] * mask_s(cell),
  mask_s = [cell >= s]
(68 slots = 17 tiles x 4 partition-groups; slots 0/67 dead -- the A_q[0]
constants ride the PSUM-evacuation bias column). Points are processed in
32-row stripes with the cell index replicated x4 along partitions, so
ONE [128,2048] tensor_scalar computes 4 knot-masks for 2 stripes, and
accumulating matmuls with block-diagonal stationaries perform 16
MAC-planes (4 knots x 4 coefs) per streamed column. The 66-knot x 4-coef
contraction (264 MACs/point) runs on the TensorEngine at 128
point-knots/cycle instead of on DVE. PSUM is evacuated via one ACT
Identity op (+A0 bias) per stripe + DMA rearrange into compact A_q
planes; final Horner on DVE.

Weights are bf16 with error-feedback (prefix-sum compensated)
quantization; ACT-generated masks use Sign (+-1) with halved weights and
their constants folded into the fp32 bias column.
"""
import numpy as np

N_POINTS = 1_000_000
N_CORES = 8
PER_CORE = N_POINTS // N_CORES  # 125000
P, F = 128, 1024  # 131072 slots >= 125000
F2 = 2 * F
NCELL = 67
NTILE = 17  # 17 tiles x 4 groups = 68 slots: 1..66 real, 0/67 dead
HALF = 512  # PSUM bank = 512 fp32
BIAS_COL = 20  # thr column holding the evac bias (A0 constants)

# engine per mask tile: 'v' = DVE is_ge(0/1), 'a' = ACT Sign(+-1).
# (GPSIMD measured ~15.7us per tile and crashes on TT -- never use it.)
MASK_ENG = ['v'] * 12 + ['a'] * 5
ACT_SLOTS = [4 * t + g for t in range(NTILE) for g in range(4)
             if MASK_ENG[t] == 'a' and 1 <= 4 * t + g <= 66]

_cache = {}


def _build_nc():
    import concourse.tile as tile
    from concourse import bacc, mybir

    fp32 = mybir.dt.float32
    bf16 = mybir.dt.bfloat16
    Alu = mybir.AluOpType
    Act = mybir.ActivationFunctionType

    nc = bacc.Bacc("TRN2", target_bir_lowering=False, debug=False,
                   num_devices=N_CORES)
    x = nc.dram_tensor("x", [P, F], fp32, kind="ExternalInput").ap()
    wts = nc.dram_tensor("w", [P, NTILE * 128], bf16,
                         kind="ExternalInput").ap()
    thr = nc.dram_tensor("thr", [P, 32], fp32, kind="ExternalInput").ap()
    y = nc.dram_tensor("y", [P, F], fp32, kind="ExternalOutput").ap()

    with tile.TileContext(nc) as tc:
        with tc.tile_pool(name="const", bufs=1) as cpool, \
             tc.tile_pool(name="data", bufs=1) as dpool, \
             tc.tile_pool(name="rep", bufs=2) as rpool, \
             tc.tile_pool(name="mask", bufs=25) as mpool, \
             tc.tile_pool(name="evac", bufs=3) as epool, \
             tc.tile_pool(name="psum", bufs=2, space="PSUM") as pspool:
            xt = dpool.tile([P, F], fp32, tag="xt")
            t1 = dpool.tile([P, F], fp32, tag="t1")
            cb = dpool.tile([P, F], bf16, tag="cb")
            nc.sync.dma_start(xt[0:64, :], x[0:64, :])
            wt = cpool.tile([P, NTILE * 128], bf16, tag="wt")
            nc.sync.dma_start(wt[:], wts)
            th = cpool.tile([P, 32], fp32, tag="th")
            nc.sync.dma_start(th[:], thr)
            nc.sync.dma_start(xt[64:128, :], x[64:128, :])
            # preprocess per row-half so pair-0 unblocks sooner:
            # t1 = 67x + (2^23 - 0.5): fp32 grid at 2^23 rounds to
            # integers -> t1 = 2^23 + round(67x - 0.5) = 2^23 + cell;
            # cb = t1 - 2^23 as bf16 (integers 0..66, exact)
            for hh in range(2):
                rr = slice(64 * hh, 64 * hh + 64)
                nc.vector.tensor_scalar(t1[rr, :], xt[rr, :], 67.0,
                                        8388607.5, Alu.mult, Alu.add)
                nc.vector.tensor_scalar(cb[rr, :], t1[rr, :], 8388608.0,
                                        None, Alu.subtract)
            # u = 67*x - cell
            u = dpool.tile([P, F], fp32, tag="u")
            nc.vector.scalar_tensor_tensor(u[:], xt[:], 67.0, cb[:],
                                           Alu.mult, Alu.subtract)
            # u^2 for the Estrin combine
            u2 = dpool.tile([P, F], fp32, tag="u2")
            nc.scalar.activation(u2[:], u[:], Act.Square)

            aq = [dpool.tile([P, F], fp32, tag=f"A{q}", name=f"A{q}")
                  for q in range(4)]
            g1 = dpool.tile([P, F], fp32, tag="g1")
            g2 = dpool.tile([P, F], fp32, tag="g2")

            # Phase 1: all rep DMAs, masks, and matmuls for both pairs --
            # keeps the in-order DVE/ACT queues from blocking pair-1 masks
            # behind pair-0 evacuation.
            accs = []
            for pair in range(2):
                s0 = 2 * pair
                # rep: [cells of stripe s0 | cells of stripe s0+1], each
                # replicated onto all four 32-partition groups
                rep = rpool.tile([P, F2], bf16, tag="rep",
                                 name=f"rep_p{pair}")
                # fill group 0 from cb, then double 32->64->128 rows
                for half in range(2):
                    nc.sync.dma_start(
                        rep[0:32, F * half:F * half + F],
                        cb[32 * (s0 + half):32 * (s0 + half) + 32, :])
                nc.sync.dma_start(rep[32:64, :], rep[0:32, :])
                nc.sync.dma_start(rep[64:128, :], rep[0:64, :])
                acc = [pspool.tile([P, F], fp32, tag=f"acc{c}",
                                   name=f"acc_p{pair}_{c}")
                       for c in range(2)]
                accs.append(acc)
                for t in range(NTILE):
                    m = mpool.tile([P, F2], bf16, tag="m",
                                   name=f"mask_p{pair}_t{t}")
                    if MASK_ENG[t] == 'v':
                        nc.vector.tensor_scalar(m[:], rep[:],
                                                th[:, t:t + 1], None,
                                                Alu.is_ge)
                    else:
                        nc.scalar.activation(m[:], rep[:], Act.Sign,
                                             bias=th[:, t:t + 1])
                    for half in range(2):
                        for c in range(2):
                            o = F * half + HALF * c
                            nc.tensor.matmul(
                                acc[half][:, HALF * c:HALF * c + HALF],
                                wt[:, 128 * t:128 * (t + 1)],
                                m[:, o:o + HALF],
                                start=(t == 0), stop=(t == NTILE - 1))

            # Phase 2: evacuate + combine, pair by pair. Evac each stripe
            # in column halves on ACT and DVE in parallel (copy PSUM->SBUF
            # adding the A0 constants from the bias column).
            bias = th[:, BIAS_COL:BIAS_COL + 1]
            for pair in range(2):
                s0 = 2 * pair
                for half in range(2):
                    s = s0 + half
                    a = accs[pair][half]
                    ev = epool.tile([P, F], fp32, tag="ev",
                                    name=f"ev_s{s}")
                    nc.scalar.activation(ev[:, 0:HALF], a[:, 0:HALF],
                                         Act.Identity, bias=bias)
                    nc.vector.tensor_scalar(ev[:, HALF:F], a[:, HALF:F],
                                            bias, None, Alu.add)
                    for q in range(4):
                        eng = nc.sync if q < 2 else nc.scalar
                        eng.dma_start(aq[q][32 * s:32 * s + 32, :],
                                      ev[32 * q:32 * q + 32, :])

                # Estrin on this pair's 64 rows while the next pair's
                # matmuls run: y = (A0 + u*A1) + u^2*(A2 + u*A3)
                rs = slice(64 * pair, 64 * pair + 64)
                nc.vector.tensor_tensor(g1[rs, :], aq[1][rs, :], u[rs, :],
                                        Alu.mult)
                nc.vector.tensor_tensor(g1[rs, :], g1[rs, :], aq[0][rs, :],
                                        Alu.add)
                nc.vector.tensor_tensor(g2[rs, :], aq[3][rs, :], u[rs, :],
                                        Alu.mult)
                nc.vector.tensor_tensor(g2[rs, :], g2[rs, :], aq[2][rs, :],
                                        Alu.add)
                nc.vector.tensor_tensor(g2[rs, :], g2[rs, :], u2[rs, :],
                                        Alu.mult)
                nc.vector.tensor_tensor(g1[rs, :], g1[rs, :], g2[rs, :],
                                        Alu.add)
                nc.sync.dma_start(y[rs, :], g1[rs, :])
    nc.compile()
    return nc


def _cell_coefs(coefs):
    """Per-cell cubic coefficients A[k, q] (float64), y = sum_q A[k,q] u^q."""
    c = np.zeros(70, dtype=np.float64)
    c[3:67] = np.asarray(coefs, dtype=np.float64)
    A = np.zeros((NCELL, 4), dtype=np.float64)
    for k in range(NCELL):
        c0, c1, c2, c3 = c[k], c[k + 1], c[k + 2], c[k + 3]
        A[k, 0] = (c0 + 4.0 * c1 + c2) / 6.0
        A[k, 1] = (-3.0 * c0 + 3.0 * c2) / 6.0
        A[k, 2] = (3.0 * c0 - 6.0 * c1 + 3.0 * c2) / 6.0
        A[k, 3] = (-c0 + 3.0 * c1 - 3.0 * c2 + c3) / 6.0
    return A


def _make_tables(coefs):
    """Build (weights [128, NTILE*128] bf16, thr [128, 32] fp32).

    Slot s (1..66) masks [cell >= s]; acc_q(cell) = sum of slot
    contributions reproduces A[cell,q] - A[0,q] to ~1e-2 via
    error-feedback bf16 quantization. ACT slots use Sign (+-1) with
    halved weights; their constants and A[0,q] fold into the fp32 evac
    bias column (thr col BIAS_COL).
    """
    import ml_dtypes

    bf = lambda v: float(np.asarray(v, dtype=ml_dtypes.bfloat16))
    A = _cell_coefs(coefs)
    act = set(ACT_SLOTS)
    st = np.zeros((68, 4), dtype=np.float64)
    for q in range(4):
        run = 0.0  # tracks acc_q(cell) - A[0,q]
        for s in range(1, NCELL):
            inc = (A[s, q] - A[0, q]) - run
            if s in act:
                w = bf(inc / 2.0)
                st[s, q] = w
                run += 2.0 * w
            else:
                w = bf(inc)
                st[s, q] = w
                run += w

    # stationary for tile t: W[32g + r', 128t + 32q + r] = st[4t+g, q]*delta
    W = np.zeros((P, NTILE * 128), dtype=np.float64)
    r = np.arange(32)
    for t in range(NTILE):
        for g in range(4):
            s = 4 * t + g
            for q in range(4):
                W[32 * g + r, 128 * t + 32 * q + r] = st[s, q]
    Wb = W.astype(ml_dtypes.bfloat16)

    thr = np.zeros((P, 32), dtype=np.float32)
    for t in range(NTILE):
        for g in range(4):
            s = 4 * t + g
            tv = 1e9 if s in (0, 67) else s - 0.5  # dead slots never fire
            if MASK_ENG[t] == 'a':
                tv = -tv  # ACT bias: sign(cell + bias)
            thr[32 * g:32 * g + 32, t] = tv
    # evac bias: A[0,q] plus the ACT-slot sign-convention constants
    for q in range(4):
        cq = A[0, q] + sum(st[s, q] for s in act)
        thr[32 * q:32 * q + 32, BIAS_COL] = np.float32(cq)
    return Wb, thr


def make_in_maps(x, coefs):
    x = np.asarray(x, dtype=np.float32)
    Wb, thr = _make_tables(coefs)
    in_maps = []
    for core in range(N_CORES):
        shard = x[core * PER_CORE:(core + 1) * PER_CORE]
        pad = np.full(P * F, 0.5, dtype=np.float32)
        pad[:PER_CORE] = shard
        in_maps.append({"x": pad.reshape(P, F), "w": Wb, "thr": thr})
    return in_maps


def kernel(x, knot_vector, coefs):
    from concourse.bass_utils import run_bass_kernel_spmd

    if "nc" not in _cache:
        _cache["nc"] = _build_nc()
    nc = _cache["nc"]

    in_maps = make_in_maps(x, coefs)
    res = run_bass_kernel_spmd(nc, in_maps, core_ids=list(range(N_CORES)))
    out = np.empty(N_POINTS, dtype=np.float32)
    for core in range(N_CORES):
        out[core * PER_CORE:(core + 1) * PER_CORE] = \
            res.results[core]["y"].reshape(-1)[:PER_CORE]
    return out


# revision 25
# speedup vs baseline: 1.0798x; 1.0798x over previous
"""B-spline evaluation kernel for Trainium2 (8 NeuronCores, data-parallel).

Math: uniform cubic B-spline, 64 basis fns, knots linspace(0,1,68).
For s = 67*x: cell = floor(s), u = s - cell,
    y = A0[cell] + A1[cell]*u + A2[cell]*u^2 + A3[cell]*u^3
with per-cell coefficients A_q derived from coefs on host.

Device algorithm (blocked PE-matmul gather):
  A_q[cell] - A_q[0] = sum_{slot s=1..66} w[s,q] * mask_s(cell),
  mask_s = [cell >= s]
(68 slots = 17 tiles x 4 partition-groups; slots 0/67 dead -- the A_q[0]
constants ride the PSUM-evacuation bias column). Points are processed in
32-row stripes with the cell index replicated x4 along partitions, so
ONE [128,2048] tensor_scalar computes 4 knot-masks for 2 stripes, and
accumulating matmuls with block-diagonal stationaries perform 16
MAC-planes (4 knots x 4 coefs) per streamed column. The 66-knot x 4-coef
contraction (264 MACs/point) runs on the TensorEngine at 128
point-knots/cycle instead of on DVE. PSUM is evacuated via one ACT
Identity op (+A0 bias) per stripe + DMA rearrange into compact A_q
planes; final Horner on DVE.

Weights are bf16 with error-feedback (prefix-sum compensated)
quantization; ACT-generated masks use Sign (+-1) with halved weights and
their constants folded into the fp32 bias column.
"""
import numpy as np

N_POINTS = 1_000_000
N_CORES = 8
PER_CORE = N_POINTS // N_CORES  # 125000
P, F = 128, 1024  # 131072 slots >= 125000
F2 = 2 * F
NCELL = 67
NTILE = 17  # 17 tiles x 4 groups = 68 slots: 1..66 real, 0/67 dead
HALF = 512  # PSUM bank = 512 fp32
BIAS_COL = 20  # thr column holding the evac bias (A0 constants)

# engine per mask tile: 'v' = DVE is_ge(0/1), 'a' = ACT Sign(+-1).
# (GPSIMD measured ~15.7us per tile and crashes on TT -- never use it.)
MASK_ENG = ['v'] * 12 + ['a'] * 5
ACT_SLOTS = [4 * t + g for t in range(NTILE) for g in range(4)
             if MASK_ENG[t] == 'a' and 1 <= 4 * t + g <= 66]

_cache = {}


def _build_nc():
    import concourse.tile as tile
    from concourse import bacc, mybir

    fp32 = mybir.dt.float32
    bf16 = mybir.dt.bfloat16
    Alu = mybir.AluOpType
    Act = mybir.ActivationFunctionType

    nc = bacc.Bacc("TRN2", target_bir_lowering=False, debug=False,
                   num_devices=N_CORES)
    x = nc.dram_tensor("x", [P, F], fp32, kind="ExternalInput").ap()
    wts = nc.dram_tensor("w", [P, NTILE * 128], bf16,
                         kind="ExternalInput").ap()
    thr = nc.dram_tensor("thr", [P, 32], fp32, kind="ExternalInput").ap()
    y = nc.dram_tensor("y", [P, F], fp32, kind="ExternalOutput").ap()

    with tile.TileContext(nc) as tc:
        with tc.tile_pool(name="const", bufs=1) as cpool, \
             tc.tile_pool(name="data", bufs=1) as dpool, \
             tc.tile_pool(name="rep", bufs=2) as rpool, \
             tc.tile_pool(name="mask", bufs=25) as mpool, \
             tc.tile_pool(name="evac", bufs=3) as epool, \
             tc.tile_pool(name="psum", bufs=2, space="PSUM") as pspool:
            xt = dpool.tile([P, F], fp32, tag="xt")
            t1 = dpool.tile([P, F], fp32, tag="t1")
            cb = dpool.tile([P, F], bf16, tag="cb")
            nc.sync.dma_start(xt[0:64, :], x[0:64, :])
            wt = cpool.tile([P, NTILE * 128], bf16, tag="wt")
            nc.sync.dma_start(wt[:], wts)
            th = cpool.tile([P, 32], fp32, tag="th")
            nc.sync.dma_start(th[:], thr)
            nc.sync.dma_start(xt[64:128, :], x[64:128, :])
            # preprocess per row-half so pair-0 unblocks sooner:
            # t1 = 67x + (2^23 - 0.5): fp32 grid at 2^23 rounds to
            # integers -> t1 = 2^23 + round(67x - 0.5) = 2^23 + cell;
            # cb = t1 - 2^23 as bf16 (integers 0..66, exact)
            for hh in range(2):
                rr = slice(64 * hh, 64 * hh + 64)
                nc.vector.tensor_scalar(t1[rr, :], xt[rr, :], 67.0,
                                        8388607.5, Alu.mult, Alu.add)
                nc.vector.tensor_scalar(cb[rr, :], t1[rr, :], 8388608.0,
                                        None, Alu.subtract)
            # u = 67*x - cell
            u = dpool.tile([P, F], fp32, tag="u")
            nc.vector.scalar_tensor_tensor(u[:], xt[:], 67.0, cb[:],
                                           Alu.mult, Alu.subtract)
            # u^2 for the Estrin combine
            u2 = dpool.tile([P, F], fp32, tag="u2")
            nc.scalar.activation(u2[:], u[:], Act.Square)

            aq = [dpool.tile([P, F], fp32, tag=f"A{q}", name=f"A{q}")
                  for q in range(4)]
            g1 = dpool.tile([P, F], fp32, tag="g1")
            g2 = dpool.tile([P, F], fp32, tag="g2")

            # Phase 1: all rep DMAs, masks, and matmuls for both pairs --
            # keeps the in-order DVE/ACT queues from blocking pair-1 masks
            # behind pair-0 evacuation.
            accs = []
            for pair in range(2):
                s0 = 2 * pair
                # rep: [cells of stripe s0 | cells of stripe s0+1], each
                # replicated onto all four 32-partition groups
                rep = rpool.tile([P, F2], bf16, tag="rep",
                                 name=f"rep_p{pair}")
                for half in range(2):
                    src = cb[32 * (s0 + half):32 * (s0 + half) + 32, :]
                    for g in range(4):
                        eng = nc.sync if g < 2 else nc.scalar
                        eng.dma_start(
                            rep[32 * g:32 * g + 32,
                                F * half:F * half + F], src)
                acc = [pspool.tile([P, F], fp32, tag=f"acc{c}",
                                   name=f"acc_p{pair}_{c}")
                       for c in range(2)]
                accs.append(acc)
                for t in range(NTILE):
                    m = mpool.tile([P, F2], bf16, tag="m",
                                   name=f"mask_p{pair}_t{t}")
                    if MASK_ENG[t] == 'v':
                        nc.vector.tensor_scalar(m[:], rep[:],
                                                th[:, t:t + 1], None,
                                                Alu.is_ge)
                    else:
                        nc.scalar.activation(m[:], rep[:], Act.Sign,
                                             bias=th[:, t:t + 1])
                    for half in range(2):
                        for c in range(2):
                            o = F * half + HALF * c
                            nc.tensor.matmul(
                                acc[half][:, HALF * c:HALF * c + HALF],
                                wt[:, 128 * t:128 * (t + 1)],
                                m[:, o:o + HALF],
                                start=(t == 0), stop=(t == NTILE - 1))

            # Phase 2: evacuate + combine, pair by pair. Evac each stripe
            # in column halves on ACT and DVE in parallel (copy PSUM->SBUF
            # adding the A0 constants from the bias column).
            bias = th[:, BIAS_COL:BIAS_COL + 1]
            for pair in range(2):
                s0 = 2 * pair
                for half in range(2):
                    s = s0 + half
                    a = accs[pair][half]
                    ev = epool.tile([P, F], fp32, tag="ev",
                                    name=f"ev_s{s}")
                    nc.scalar.activation(ev[:, 0:HALF], a[:, 0:HALF],
                                         Act.Identity, bias=bias)
                    nc.vector.tensor_scalar(ev[:, HALF:F], a[:, HALF:F],
                                            bias, None, Alu.add)
                    for q in range(4):
                        eng = nc.sync if q < 2 else nc.scalar
                        eng.dma_start(aq[q][32 * s:32 * s + 32, :],
                                      ev[32 * q:32 * q + 32, :])

                # Estrin on this pair's 64 rows while the next pair's
                # matmuls run: y = (A0 + u*A1) + u^2*(A2 + u*A3)
                rs = slice(64 * pair, 64 * pair + 64)
                nc.vector.tensor_tensor(g1[rs, :], aq[1][rs, :], u[rs, :],
                                        Alu.mult)
                nc.vector.tensor_tensor(g2[rs, :], aq[3][rs, :], u[rs, :],
                                        Alu.mult)
                nc.vector.tensor_tensor(g1[rs, :], g1[rs, :], aq[0][rs, :],
                                        Alu.add)
                nc.vector.tensor_tensor(g2[rs, :], g2[rs, :], aq[2][rs, :],
                                        Alu.add)
                nc.vector.tensor_tensor(g2[rs, :], g2[rs, :], u2[rs, :],
                                        Alu.mult)
                nc.vector.tensor_tensor(g1[rs, :], g1[rs, :], g2[rs, :],
                                        Alu.add)
                nc.sync.dma_start(y[rs, :], g1[rs, :])
    nc.compile()
    return nc


def _cell_coefs(coefs):
    """Per-cell cubic coefficients A[k, q] (float64), y = sum_q A[k,q] u^q."""
    c = np.zeros(70, dtype=np.float64)
    c[3:67] = np.asarray(coefs, dtype=np.float64)
    A = np.zeros((NCELL, 4), dtype=np.float64)
    for k in range(NCELL):
        c0, c1, c2, c3 = c[k], c[k + 1], c[k + 2], c[k + 3]
        A[k, 0] = (c0 + 4.0 * c1 + c2) / 6.0
        A[k, 1] = (-3.0 * c0 + 3.0 * c2) / 6.0
        A[k, 2] = (3.0 * c0 - 6.0 * c1 + 3.0 * c2) / 6.0
        A[k, 3] = (-c0 + 3.0 * c1 - 3.0 * c2 + c3) / 6.0
    return A


def _make_tables(coefs):
    """Build (weights [128, NTILE*128] bf16, thr [128, 32] fp32).

    Slot s (1..66) masks [cell >= s]; acc_q(cell) = sum of slot
    contributions reproduces A[cell,q] - A[0,q] to ~1e-2 via
    error-feedback bf16 quantization. ACT slots use Sign (+-1) with
    halved weights; their constants and A[0,q] fold into the fp32 evac
    bias column (thr col BIAS_COL).
    """
    import ml_dtypes

    bf = lambda v: float(np.asarray(v, dtype=ml_dtypes.bfloat16))
    A = _cell_coefs(coefs)
    act = set(ACT_SLOTS)
    st = np.zeros((68, 4), dtype=np.float64)
    for q in range(4):
        run = 0.0  # tracks acc_q(cell) - A[0,q]
        for s in range(1, NCELL):
            inc = (A[s, q] - A[0, q]) - run
            if s in act:
                w = bf(inc / 2.0)
                st[s, q] = w
                run += 2.0 * w
            else:
                w = bf(inc)
                st[s, q] = w
                run += w

    # stationary for tile t: W[32g + r', 128t + 32q + r] = st[4t+g, q]*delta
    W = np.zeros((P, NTILE * 128), dtype=np.float64)
    r = np.arange(32)
    for t in range(NTILE):
        for g in range(4):
            s = 4 * t + g
            for q in range(4):
                W[32 * g + r, 128 * t + 32 * q + r] = st[s, q]
    Wb = W.astype(ml_dtypes.bfloat16)

    thr = np.zeros((P, 32), dtype=np.float32)
    for t in range(NTILE):
        for g in range(4):
            s = 4 * t + g
            tv = 1e9 if s in (0, 67) else s - 0.5  # dead slots never fire
            if MASK_ENG[t] == 'a':
                tv = -tv  # ACT bias: sign(cell + bias)
            thr[32 * g:32 * g + 32, t] = tv
    # evac bias: A[0,q] plus the ACT-slot sign-convention constants
    for q in range(4):
        cq = A[0, q] + sum(st[s, q] for s in act)
        thr[32 * q:32 * q + 32, BIAS_COL] = np.float32(cq)
    return Wb, thr


def make_in_maps(x, coefs):
    x = np.asarray(x, dtype=np.float32)
    Wb, thr = _make_tables(coefs)
    in_maps = []
    for core in range(N_CORES):
        shard = x[core * PER_CORE:(core + 1) * PER_CORE]
        pad = np.full(P * F, 0.5, dtype=np.float32)
        pad[:PER_CORE] = shard
        in_maps.append({"x": pad.reshape(P, F), "w": Wb, "thr": thr})
    return in_maps


def kernel(x, knot_vector, coefs):
    from concourse.bass_utils import run_bass_kernel_spmd

    if "nc" not in _cache:
        _cache["nc"] = _build_nc()
    nc = _cache["nc"]

    in_maps = make_in_maps(x, coefs)
    res = run_bass_kernel_spmd(nc, in_maps, core_ids=list(range(N_CORES)))
    out = np.empty(N_POINTS, dtype=np.float32)
    for core in range(N_CORES):
        out[core * PER_CORE:(core + 1) * PER_CORE] = \
            res.results[core]["y"].reshape(-1)[:PER_CORE]
    return out


# revision 28
# speedup vs baseline: 1.0852x; 1.0051x over previous
"""B-spline evaluation kernel for Trainium2 (8 NeuronCores, data-parallel).

Math: uniform cubic B-spline, 64 basis fns, knots linspace(0,1,68).
For s = 67*x: cell = floor(s), u = s - cell,
    y = A0[cell] + A1[cell]*u + A2[cell]*u^2 + A3[cell]*u^3
with per-cell coefficients A_q derived from coefs on host.

Device algorithm (blocked PE-matmul gather):
  A_q[cell] - A_q[0] = sum_{slot s=1..66} w[s,q] * mask_s(cell),
  mask_s = [cell >= s]
(68 slots = 17 tiles x 4 partition-groups; slots 0/67 dead -- the A_q[0]
constants ride the PSUM-evacuation bias column). Points are processed in
32-row stripes with the cell index replicated x4 along partitions, so
ONE [128,2048] tensor_scalar computes 4 knot-masks for 2 stripes, and
accumulating matmuls with block-diagonal stationaries perform 16
MAC-planes (4 knots x 4 coefs) per streamed column. The 66-knot x 4-coef
contraction (264 MACs/point) runs on the TensorEngine at 128
point-knots/cycle instead of on DVE. PSUM is evacuated via one ACT
Identity op (+A0 bias) per stripe + DMA rearrange into compact A_q
planes; final Horner on DVE.

Weights are bf16 with error-feedback (prefix-sum compensated)
quantization; ACT-generated masks use Sign (+-1) with halved weights and
their constants folded into the fp32 bias column.
"""
import numpy as np

N_POINTS = 1_000_000
N_CORES = 8
PER_CORE = N_POINTS // N_CORES  # 125000
P, F = 128, 1024  # 131072 slots >= 125000
F2 = 2 * F
NCELL = 67
NTILE = 17  # 17 tiles x 4 groups = 68 slots: 1..66 real, 0/67 dead
HALF = 512  # PSUM bank = 512 fp32
BIAS_COL = 20  # thr column holding the evac bias (A0 constants)

# engine per mask tile: 'v' = DVE is_ge(0/1), 'a' = ACT Sign(+-1).
# (GPSIMD measured ~15.7us per tile and crashes on TT -- never use it.)
MASK_ENG = ['v'] * 13 + ['a'] * 4
ACT_SLOTS = [4 * t + g for t in range(NTILE) for g in range(4)
             if MASK_ENG[t] == 'a' and 1 <= 4 * t + g <= 66]

_cache = {}


def _build_nc():
    import concourse.tile as tile
    from concourse import bacc, mybir

    fp32 = mybir.dt.float32
    bf16 = mybir.dt.bfloat16
    Alu = mybir.AluOpType
    Act = mybir.ActivationFunctionType

    nc = bacc.Bacc("TRN2", target_bir_lowering=False, debug=False,
                   num_devices=N_CORES)
    x = nc.dram_tensor("x", [P, F], fp32, kind="ExternalInput").ap()
    wts = nc.dram_tensor("w", [P, NTILE * 128], bf16,
                         kind="ExternalInput").ap()
    thr = nc.dram_tensor("thr", [P, 32], fp32, kind="ExternalInput").ap()
    y = nc.dram_tensor("y", [P, F], fp32, kind="ExternalOutput").ap()

    with tile.TileContext(nc) as tc:
        with tc.tile_pool(name="const", bufs=1) as cpool, \
             tc.tile_pool(name="data", bufs=1) as dpool, \
             tc.tile_pool(name="rep", bufs=2) as rpool, \
             tc.tile_pool(name="mask", bufs=25) as mpool, \
             tc.tile_pool(name="evac", bufs=3) as epool, \
             tc.tile_pool(name="psum", bufs=2, space="PSUM") as pspool:
            xt = dpool.tile([P, F], fp32, tag="xt")
            t1 = dpool.tile([P, F], fp32, tag="t1")
            cb = dpool.tile([P, F], bf16, tag="cb")
            nc.sync.dma_start(xt[0:64, :], x[0:64, :])
            wt = cpool.tile([P, NTILE * 128], bf16, tag="wt")
            nc.sync.dma_start(wt[:], wts)
            th = cpool.tile([P, 32], fp32, tag="th")
            nc.sync.dma_start(th[:], thr)
            nc.sync.dma_start(xt[64:128, :], x[64:128, :])
            # preprocess per row-half so pair-0 unblocks sooner:
            # t1 = 67x + (2^23 - 0.5): fp32 grid at 2^23 rounds to
            # integers -> t1 = 2^23 + round(67x - 0.5) = 2^23 + cell;
            # cb = t1 - 2^23 as bf16 (integers 0..66, exact)
            for hh in range(2):
                rr = slice(64 * hh, 64 * hh + 64)
                nc.vector.tensor_scalar(t1[rr, :], xt[rr, :], 67.0,
                                        8388607.5, Alu.mult, Alu.add)
                nc.vector.tensor_scalar(cb[rr, :], t1[rr, :], 8388608.0,
                                        None, Alu.subtract)
            # u = 67*x - cell
            u = dpool.tile([P, F], fp32, tag="u")
            nc.vector.scalar_tensor_tensor(u[:], xt[:], 67.0, cb[:],
                                           Alu.mult, Alu.subtract)
            # u^2 for the Estrin combine
            u2 = dpool.tile([P, F], fp32, tag="u2")
            nc.scalar.activation(u2[:], u[:], Act.Square)

            aq = [dpool.tile([P, F], fp32, tag=f"A{q}", name=f"A{q}")
                  for q in range(4)]
            g1 = dpool.tile([P, F], fp32, tag="g1")
            g2 = dpool.tile([P, F], fp32, tag="g2")

            # Phase 1: all rep DMAs, masks, and matmuls for both pairs --
            # keeps the in-order DVE/ACT queues from blocking pair-1 masks
            # behind pair-0 evacuation.
            accs = []
            for pair in range(2):
                s0 = 2 * pair
                # rep: [cells of stripe s0 | cells of stripe s0+1], each
                # replicated onto all four 32-partition groups
                rep = rpool.tile([P, F2], bf16, tag="rep",
                                 name=f"rep_p{pair}")
                for half in range(2):
                    src = cb[32 * (s0 + half):32 * (s0 + half) + 32, :]
                    for g in range(4):
                        # pair 0 is head-latency-critical: split across
                        # queues; pair 1 stays off ACT's queue so its
                        # Sign-masks and evacs aren't delayed
                        eng = nc.scalar if (g >= 2 and pair == 0) \
                            else nc.sync
                        eng.dma_start(
                            rep[32 * g:32 * g + 32,
                                F * half:F * half + F], src)
                acc = [pspool.tile([P, F], fp32, tag=f"acc{c}",
                                   name=f"acc_p{pair}_{c}")
                       for c in range(2)]
                accs.append(acc)
                for t in range(NTILE):
                    m = mpool.tile([P, F2], bf16, tag="m",
                                   name=f"mask_p{pair}_t{t}")
                    if MASK_ENG[t] == 'v':
                        nc.vector.tensor_scalar(m[:], rep[:],
                                                th[:, t:t + 1], None,
                                                Alu.is_ge)
                    else:
                        nc.scalar.activation(m[:], rep[:], Act.Sign,
                                             bias=th[:, t:t + 1])
                    for half in range(2):
                        for c in range(2):
                            o = F * half + HALF * c
                            nc.tensor.matmul(
                                acc[half][:, HALF * c:HALF * c + HALF],
                                wt[:, 128 * t:128 * (t + 1)],
                                m[:, o:o + HALF],
                                start=(t == 0), stop=(t == NTILE - 1))

            # Phase 2: evacuate + combine, pair by pair. Evac each stripe
            # in column halves on ACT and DVE in parallel (copy PSUM->SBUF
            # adding the A0 constants from the bias column).
            bias = th[:, BIAS_COL:BIAS_COL + 1]
            for pair in range(2):
                s0 = 2 * pair
                for half in range(2):
                    s = s0 + half
                    a = accs[pair][half]
                    ev = epool.tile([P, F], fp32, tag="ev",
                                    name=f"ev_s{s}")
                    nc.scalar.activation(ev[:, 0:HALF], a[:, 0:HALF],
                                         Act.Identity, bias=bias)
                    nc.vector.tensor_scalar(ev[:, HALF:F], a[:, HALF:F],
                                            bias, None, Alu.add)
                    # q-order matches Estrin consumption (A1/A3 first)
                    for q in (1, 0, 3, 2):
                        eng = nc.sync if q < 2 else nc.scalar
                        eng.dma_start(aq[q][32 * s:32 * s + 32, :],
                                      ev[32 * q:32 * q + 32, :])

                # Estrin on this pair's 64 rows while the next pair's
                # matmuls run: y = (A0 + u*A1) + u^2*(A2 + u*A3)
                rs = slice(64 * pair, 64 * pair + 64)
                nc.vector.tensor_tensor(g1[rs, :], aq[1][rs, :], u[rs, :],
                                        Alu.mult)
                nc.vector.tensor_tensor(g2[rs, :], aq[3][rs, :], u[rs, :],
                                        Alu.mult)
                nc.vector.tensor_tensor(g1[rs, :], g1[rs, :], aq[0][rs, :],
                                        Alu.add)
                nc.vector.tensor_tensor(g2[rs, :], g2[rs, :], aq[2][rs, :],
                                        Alu.add)
                nc.vector.tensor_tensor(g2[rs, :], g2[rs, :], u2[rs, :],
                                        Alu.mult)
                nc.vector.tensor_tensor(g1[rs, :], g1[rs, :], g2[rs, :],
                                        Alu.add)
                nc.sync.dma_start(y[rs, :], g1[rs, :])
    nc.compile()
    return nc


def _cell_coefs(coefs):
    """Per-cell cubic coefficients A[k, q] (float64), y = sum_q A[k,q] u^q."""
    c = np.zeros(70, dtype=np.float64)
    c[3:67] = np.asarray(coefs, dtype=np.float64)
    A = np.zeros((NCELL, 4), dtype=np.float64)
    for k in range(NCELL):
        c0, c1, c2, c3 = c[k], c[k + 1], c[k + 2], c[k + 3]
        A[k, 0] = (c0 + 4.0 * c1 + c2) / 6.0
        A[k, 1] = (-3.0 * c0 + 3.0 * c2) / 6.0
        A[k, 2] = (3.0 * c0 - 6.0 * c1 + 3.0 * c2) / 6.0
        A[k, 3] = (-c0 + 3.0 * c1 - 3.0 * c2 + c3) / 6.0
    return A


def _make_tables(coefs):
    """Build (weights [128, NTILE*128] bf16, thr [128, 32] fp32).

    Slot s (1..66) masks [cell >= s]; acc_q(cell) = sum of slot
    contributions reproduces A[cell,q] - A[0,q] to ~1e-2 via
    error-feedback bf16 quantization. ACT slots use Sign (+-1) with
    halved weights; their constants and A[0,q] fold into the fp32 evac
    bias column (thr col BIAS_COL).
    """
    import ml_dtypes

    bf = lambda v: float(np.asarray(v, dtype=ml_dtypes.bfloat16))
    A = _cell_coefs(coefs)
    act = set(ACT_SLOTS)
    st = np.zeros((68, 4), dtype=np.float64)
    for q in range(4):
        run = 0.0  # tracks acc_q(cell) - A[0,q]
        for s in range(1, NCELL):
            inc = (A[s, q] - A[0, q]) - run
            if s in act:
                w = bf(inc / 2.0)
                st[s, q] = w
                run += 2.0 * w
            else:
                w = bf(inc)
                st[s, q] = w
                run += w

    # stationary for tile t: W[32g + r', 128t + 32q + r] = st[4t+g, q]*delta
    W = np.zeros((P, NTILE * 128), dtype=np.float64)
    r = np.arange(32)
    for t in range(NTILE):
        for g in range(4):
            s = 4 * t + g
            for q in range(4):
                W[32 * g + r, 128 * t + 32 * q + r] = st[s, q]
    Wb = W.astype(ml_dtypes.bfloat16)

    thr = np.zeros((P, 32), dtype=np.float32)
    for t in range(NTILE):
        for g in range(4):
            s = 4 * t + g
            tv = 1e9 if s in (0, 67) else s - 0.5  # dead slots never fire
            if MASK_ENG[t] == 'a':
                tv = -tv  # ACT bias: sign(cell + bias)
            thr[32 * g:32 * g + 32, t] = tv
    # evac bias: A[0,q] plus the ACT-slot sign-convention constants
    for q in range(4):
        cq = A[0, q] + sum(st[s, q] for s in act)
        thr[32 * q:32 * q + 32, BIAS_COL] = np.float32(cq)
    return Wb, thr


def make_in_maps(x, coefs):
    x = np.asarray(x, dtype=np.float32)
    Wb, thr = _make_tables(coefs)
    in_maps = []
    for core in range(N_CORES):
        shard = x[core * PER_CORE:(core + 1) * PER_CORE]
        pad = np.full(P * F, 0.5, dtype=np.float32)
        pad[:PER_CORE] = shard
        in_maps.append({"x": pad.reshape(P, F), "w": Wb, "thr": thr})
    return in_maps


def kernel(x, knot_vector, coefs):
    from concourse.bass_utils import run_bass_kernel_spmd

    if "nc" not in _cache:
        _cache["nc"] = _build_nc()
    nc = _cache["nc"]

    in_maps = make_in_maps(x, coefs)
    res = run_bass_kernel_spmd(nc, in_maps, core_ids=list(range(N_CORES)))
    out = np.empty(N_POINTS, dtype=np.float32)
    for core in range(N_CORES):
        out[core * PER_CORE:(core + 1) * PER_CORE] = \
            res.results[core]["y"].reshape(-1)[:PER_CORE]
    return out
